# revision 1
# baseline (speedup 1.0000x reference)
"""Trainium2 Bass kernel for the EnforcedNeuralODE recurrence.

Reference computation (per timestep):
    x_t = fc_w @ concat(x_{t-1}, f_{t-1}) + fc_b
      i.e. x_t = Wx x_{t-1} + Wf f_{t-1} + b
over T-1 = 4095 steps, batch 256, state 64, force 64.
Output: [T, B, 64] = concat([x_0], [x_1..x_{T-1}]).

Strategy: data-parallel batch shard (32 samples/core across 8 cores); on
each core a blocked parallel scan over K=32-step blocks:
  P1: within-block prefixes (odd steps only, unroll-2 chain), batched
      across the chunk's blocks in the matmul free dim:
        h_{2p+1} = Wx^2 h_{2p-1} + (Wx Wf) f_{2p} + Wf f_{2p+1} + (Wx b + b)
  P2: block-boundary scan (128 sequential tiny steps):
        s_{b+1} = Wx^K s_b + h_{K-1}
  P3: combine (embarrassingly parallel):
        even j: X_j = Wx^{j+1} s + Wx h_{j-1} + Wf f_j + b
        odd  j: X_j = Wx^{j+1} s + h_j
Matrix powers/products precomputed on host (f64, cast f32).

Hardware constraints honored:
  - Every matmul uses contraction rows 0..64 (K=65; row 64 is a host-
    provided zeros row for F/H, a ones row for S so the bias rides the
    matmul as lhsT row 64).  Mixing operand partition-halves between
    matmuls that share PSUM partitions crashes the device
    (NRT_EXEC_UNIT_UNRECOVERABLE), so everything stays on one half;
    uniform (rows, 64-col) tiling mode also avoids PE drain thrash.
  - PSUM column tiles (0,0)/(0,64) pack even/odd steps into one
    [128, N] psum tile so the PSUM->SBUF evacuation runs 128 wide.
"""

import numpy as np
from contextlib import ExitStack

NCORES = 8
BATCH, STATE, FDIM, TIMESPAN = 256, 64, 64, 4096

# per-core tiling
BC = BATCH // NCORES        # 32 batch per core
K = 32                      # steps per block
PAIRS = K // 2              # 16
NB = TIMESPAN // K          # 128 blocks (steps padded 4095 -> 4096)
NBC = 8                     # blocks per chunk
CHUNKS = NB // NBC          # 16
N = NBC * BC                # 256 free-dim per step column
F_COLS = PAIRS * 2 * N      # 8192 forcing cols per chunk (both parities)
H_COLS = PAIRS * N          # 4096 prefix cols per chunk
O_COLS = PAIRS * N          # 4096 output cols per chunk (pair-packed)

_NC_CACHE: dict = {}

# matmul operand dtype: "float32r" runs the PE at 1 cycle/row (vs 4 for
# float32) at ~1.5e-4 per-matmul relative error (TF32-like). Outputs and
# PSUM accumulation stay full fp32 either way.
MM_DTYPE = "bfloat16"


def _set_dims(ncores=8, bc=32, k=32, nbc=8, chunks=16):
    """Override problem dims (testing only). Recomputes derived globals."""
    global NCORES, BATCH, BC, K, PAIRS, NB, NBC, CHUNKS, N
    global F_COLS, H_COLS, O_COLS, TIMESPAN
    NCORES, BC, K, NBC, CHUNKS = ncores, bc, k, nbc, chunks
    BATCH = NCORES * BC
    PAIRS = K // 2
    NB = CHUNKS * NBC
    TIMESPAN = NB * K
    N = NBC * BC
    F_COLS = PAIRS * 2 * N
    H_COLS = PAIRS * N
    O_COLS = PAIRS * N


def _build_nc(chunks, nbc, bc, k):
    """Build + compile the per-core Bass module (SPMD: same NEFF all cores)."""
    import concourse.bass as bass  # noqa: F401
    import concourse.tile as tile
    from concourse import bacc, mybir

    pairs = k // 2
    n = nbc * bc
    f_cols = pairs * 2 * n
    h_cols = pairs * n
    o_cols = pairs * n
    nb = chunks * nbc
    f32 = mybir.dt.float32
    mdt = getattr(mybir.dt, MM_DTYPE)
    AF = mybir.ActivationFunctionType

    nc = bacc.Bacc("TRN2", target_bir_lowering=False, debug=False)

    f_dram = nc.dram_tensor("f", [65, chunks * f_cols], mdt, kind="ExternalInput")
    wpow_dram = nc.dram_tensor("wpow", [65, k * 64], mdt, kind="ExternalInput")
    wsml_dram = nc.dram_tensor("wsml", [65, 720], mdt, kind="ExternalInput")
    s0_dram = nc.dram_tensor("s0", [65, (nb + 1) * bc], mdt, kind="ExternalInput")
    zrow_dram = nc.dram_tensor("zrow", [1, h_cols], mdt, kind="ExternalInput")
    out_dram = nc.dram_tensor("out", [128, chunks * o_cols], f32, kind="ExternalOutput")

    with tile.TileContext(nc) as tc, ExitStack() as ctx:
        singles = ctx.enter_context(tc.tile_pool(name="singles", bufs=1))
        fpool = ctx.enter_context(tc.tile_pool(name="fpool", bufs=4))
        hpool = ctx.enter_context(tc.tile_pool(name="hpool", bufs=2))
        opool = ctx.enter_context(tc.tile_pool(name="opool", bufs=3))
        p1ps = ctx.enter_context(tc.tile_pool(name="p1ps", bufs=2, space="PSUM"))
        p3ps = ctx.enter_context(tc.tile_pool(name="p3ps", bufs=4, space="PSUM"))
        p2ps = ctx.enter_context(tc.tile_pool(name="p2ps", bufs=2, space="PSUM"))

        wpow = singles.tile([65, k * 64], mdt)
        nc.sync.dma_start(out=wpow[:], in_=wpow_dram[:])
        wsml = singles.tile([65, 720], mdt)
        nc.sync.dma_start(out=wsml[:], in_=wsml_dram[:])
        # block start states: [65, (nb+1)*bc]; row 64 = ones (bias row),
        # cols 0:bc = x0^T, rest zeros -- all host-provided (f32r memset is
        # invalid ISA)
        s65 = singles.tile([65, (nb + 1) * bc], mdt)
        nc.sync.dma_start(out=s65[:], in_=s0_dram[:])

        # weight slices inside wsml (columns); row 64 zero everywhere.
        # f32r matmuls must write psum col base 0, so P3 extras use M=128
        # zero-padded weights ([Wx|0] etc) to keep one 128-wide psum tile.
        a_wx2 = wsml[:, 0:64]        # (Wx^2)^T            [65, 64]
        a_wxwf = wsml[:, 64:128]     # (Wx Wf)^T           [65, 64]
        a_wf = wsml[:, 192:256]      # Wf^T                [65, 64]
        a_eye = wsml[:, 256:320]     # I                   [65, 64]
        b2_ap = wsml[0:64, 320:321]  # Wx b + b (bias for P1 copies)
        a_wx128 = wsml[:, 336:464]   # [Wx^T | 0]          [65, 128]
        a_wf128 = wsml[:, 464:592]   # [Wf^T | 0]          [65, 128]
        a_eye128 = wsml[:, 592:720]  # [0 | I]             [65, 128]

        fh_cols = f_cols // 2      # forcing cols per half-chunk tile
        fh_pairs = pairs // 2      # pairs per F tile
        os_pairs = pairs // 2      # pairs per out-stage tile
        os_cols = os_pairs * n

        for c in range(chunks):
            ftiles = []
            for fh in range(2):
                ft = fpool.tile([65, fh_cols], mdt, tag="F")
                nc.sync.dma_start(
                    out=ft[:],
                    in_=f_dram[:, c * f_cols + fh * fh_cols : c * f_cols + (fh + 1) * fh_cols],
                )
                ftiles.append(ft)

            htile = hpool.tile([65, h_cols], mdt, tag="H")
            nc.sync.dma_start(out=htile[64:65, :], in_=zrow_dram[:])

            def fslice(p, parity):
                ft = ftiles[p // fh_pairs]
                base = (p % fh_pairs) * 2 * n + parity * n
                return ft[:, base : base + n]

            # ---- P1: within-block odd prefixes (sequential chain) ----
            for p in range(pairs):
                ps = p1ps.tile([64, n], f32)
                nc.tensor.matmul(ps[:], a_wxwf, fslice(p, 0), start=True, stop=False)
                if p > 0:
                    nc.tensor.matmul(
                        ps[:], a_wx2, htile[:, (p - 1) * n : p * n],
                        start=False, stop=False,
                    )
                nc.tensor.matmul(ps[:], a_wf, fslice(p, 1), start=False, stop=True)
                # h = psum + b2   (ScalarE, PSUM->SBUF with per-partition bias)
                nc.scalar.activation(
                    htile[0:64, p * n : (p + 1) * n], ps[:], AF.Identity, bias=b2_ap
                )

            # ---- P2: block-boundary scan for this chunk's blocks ----
            for blk in range(nbc):
                bg = c * nbc + blk
                ps2 = p2ps.tile([64, bc], f32)
                nc.tensor.matmul(
                    ps2[:],
                    wpow[:, (k - 1) * 64 : k * 64],
                    s65[:, bg * bc : (bg + 1) * bc],
                    start=True, stop=False,
                )
                nc.tensor.matmul(
                    ps2[:],
                    a_eye,
                    htile[:, (pairs - 1) * n + blk * bc : (pairs - 1) * n + (blk + 1) * bc],
                    start=False, stop=True,
                )
                nc.scalar.activation(
                    s65[0:64, (bg + 1) * bc : (bg + 2) * bc], ps2[:], AF.Copy
                )

            # ---- P3: combine + write out ----
            scol = s65[:, c * n : (c + 1) * n]
            for ohalf in range(2):
                ostage = opool.tile([128, os_cols], f32, tag="OS")
                for pp in range(os_pairs):
                    p = ohalf * os_pairs + pp
                    j0, j1 = 2 * p, 2 * p + 1
                    px = p3ps.tile([128, n], f32)
                    # both steps' s-terms in one M=128 matmul (wpow cols are
                    # contiguous for the pair); even extras -> psum 0:64,
                    # odd extra -> psum 64:128
                    nc.tensor.matmul(
                        px[:, :], wpow[:, j0 * 64 : (j1 + 1) * 64], scol,
                        start=True, stop=False,
                    )
                    if p > 0:
                        nc.tensor.matmul(
                            px[:, :], a_wx128, htile[:, (p - 1) * n : p * n],
                            start=False, stop=False,
                        )
                    nc.tensor.matmul(
                        px[:, :], a_wf128, fslice(p, 0), start=False, stop=False
                    )
                    nc.tensor.matmul(
                        px[:, :], a_eye128, htile[:, p * n : (p + 1) * n],
                        start=False, stop=True,
                    )
                    nc.vector.tensor_copy(ostage[:, pp * n : (pp + 1) * n], px[:])
                nc.sync.dma_start(
                    out=out_dram[:, c * o_cols + ohalf * os_cols : c * o_cols + (ohalf + 1) * os_cols],
                    in_=ostage[:],
                )

    nc.compile()
    return nc


def _get_nc():
    key = (CHUNKS, NBC, BC, K)
    if key not in _NC_CACHE:
        _NC_CACHE[key] = _build_nc(CHUNKS, NBC, BC, K)
    return _NC_CACHE[key]


def _host_prep(inputs, forcing, fc_w, fc_b):
    """Build per-core input maps (numpy only, untimed)."""
    S = STATE
    fc_w = np.asarray(fc_w, np.float32)
    fc_b = np.asarray(fc_b, np.float32)
    Wx = fc_w[:, :S].astype(np.float64)
    Wf = fc_w[:, S:].astype(np.float64)
    b = fc_b.astype(np.float64)

    wsml = np.zeros((65, 720), np.float32)
    wsml[0:64, 0:64] = (Wx @ Wx).T.astype(np.float32)
    wsml[0:64, 64:128] = (Wx @ Wf).T.astype(np.float32)
    wsml[0:64, 192:256] = Wf.T.astype(np.float32)
    wsml[0:64, 256:320] = np.eye(64, dtype=np.float32)
    wsml[0:64, 320] = (Wx @ b + b).astype(np.float32)
    wsml[0:64, 336:400] = Wx.T.astype(np.float32)   # [Wx|0] left half
    wsml[0:64, 464:528] = Wf.T.astype(np.float32)   # [Wf|0] left half
    wsml[0:64, 656:720] = np.eye(64, dtype=np.float32)  # [0|I] right half

    # wpow: col block j holds (Wx^{j+1})^T; row 64 = b for even j else 0
    wpow = np.zeros((65, K * 64), np.float32)
    P = np.eye(S, dtype=np.float64)
    for j in range(K):
        P = Wx @ P
        wpow[0:64, j * 64 : (j + 1) * 64] = P.T.astype(np.float32)
        if j % 2 == 0:
            wpow[64, j * 64 : (j + 1) * 64] = b.astype(np.float32)

    # forcing: [T-1, B, F] -> pad -> [65, c, p, parity, blk, bcore] cols
    steps = TIMESPAN
    fpad = np.zeros((steps, BATCH, FDIM), np.float32)
    fpad[: TIMESPAN - 1] = np.asarray(forcing, np.float32)
    # t = (c*NBC + blk)*K + 2p + parity
    arr = fpad.reshape(CHUNKS, NBC, PAIRS, 2, BATCH, FDIM)
    arr = arr.transpose(5, 0, 2, 3, 1, 4)  # [feat, c, p, parity, blk, bfull]

    inputs = np.asarray(inputs, np.float32)
    if MM_DTYPE == "bfloat16":
        import ml_dtypes

        mm_np = ml_dtypes.bfloat16
    else:
        mm_np = np.float32
    zrow = np.zeros((1, H_COLS), mm_np)
    km_nb = NB
    wpow = wpow.astype(mm_np)
    wsml = wsml.astype(mm_np)
    in_maps = []
    for core in range(NCORES):
        bs = slice(core * BC, (core + 1) * BC)
        fcore = np.zeros((65, CHUNKS * F_COLS), mm_np)
        fcore[0:64] = (
            np.ascontiguousarray(arr[..., bs]).reshape(64, CHUNKS * F_COLS).astype(mm_np)
        )
        s0 = np.zeros((65, (km_nb + 1) * BC), mm_np)
        s0[64, :] = 1.0
        s0[0:64, 0:BC] = inputs[bs].T.astype(mm_np)
        in_maps.append(
            {"f": fcore, "wpow": wpow, "wsml": wsml, "s0": s0, "zrow": zrow}
        )
    return in_maps


def _host_decode(results, inputs):
    """Per-core out [128, CHUNKS*O_COLS] -> full [T, B, S]."""
    inputs = np.asarray(inputs, np.float32)
    out = np.empty((TIMESPAN, BATCH, STATE), np.float32)
    out[0] = inputs
    for core in range(NCORES):
        o = results[core]["out"].reshape(2, 64, CHUNKS, PAIRS, NBC, BC)
        # [parity, s, c, p, blk, b] -> [c, blk, p, parity, b, s]
        o = o.transpose(2, 4, 3, 0, 5, 1).reshape(TIMESPAN, BC, STATE)
        out[1:, core * BC : (core + 1) * BC] = o[: TIMESPAN - 1]
    return out


def kernel(inputs, forcing, fc_w, fc_b, timespan):
    from concourse.bass_utils import run_bass_kernel_spmd

    timespan = int(timespan)
    assert timespan == TIMESPAN, f"hardcoded for timespan={TIMESPAN}, got {timespan}"
    nc = _get_nc()
    in_maps = _host_prep(inputs, forcing, fc_w, fc_b)
    res = run_bass_kernel_spmd(nc, in_maps, core_ids=list(range(NCORES)))
    return _host_decode(res.results, inputs)


if __name__ == "__main__":
    nc = _get_nc()
    print("built ok")



# revision 3
# speedup vs baseline: 2.1349x; 2.1349x over previous
"""Trainium2 Bass kernel for the EnforcedNeuralODE recurrence.

Reference computation (per timestep):
    x_t = fc_w @ concat(x_{t-1}, f_{t-1}) + fc_b
      i.e. x_t = Wx x_{t-1} + Wf f_{t-1} + b
over T-1 = 4095 steps, batch 256, state 64, force 64.
Output: [T, B, 64] = concat([x_0], [x_1..x_{T-1}]).

Strategy: data-parallel batch shard (32 samples/core across 8 cores); on
each core a blocked parallel scan over K=32-step blocks.  All matmuls
contract over the full 128 partitions and pack the two parities of a
step-pair into the two PSUM partition halves (even steps rows 0:64, odd
rows 64:128), so every PE stream does maximal work per column:

  P1: pair chain producing BOTH parities of the within-block prefix
      (bias included), batched across the chunk's blocks in the free dim:
        [h_2p; h_2p+1] = W1a.[f_2p; f_2p+1] + W1b.[.; h_2p-1]  (+bias)
      where W1a = [[Wf, 0], [WxWf, Wf]], W1b = [[0, Wx], [0, Wx^2]]
      (block layout; lhsT stored transposed), 2 matmuls per 2 steps.
  P2: block-boundary scan s_{b+1} = Wx^K s_b + h_{K-1}: two small
      128-row matmuls per block, interleaved between P1 pairs in
      program order so the tensor engine never stalls on the
      scalar-engine s-copy ping-pong.
  P3: combine: one matmul  [Wx^{2p+1}; Wx^{2p+2}] . s  per pair, then a
      VectorE tensor_add of the prefix h-pair (PSUM + SBUF -> SBUF),
      pipelined one chunk behind P1 so it overlaps the next chunk's
      prefix work.  Matrix powers/products precomputed on host (f64,
      cast bf16).

Hardware notes:
  - Every matmul reads operand partitions 0..128 (unused halves are
    host-zeroed), avoiding the mixed-operand-partition-half PE crash
    (NRT_EXEC_UNIT_UNRECOVERABLE) seen when accumulating matmuls whose
    operands sit on different halves.
  - bf16 operands run the PE at 1 row/cycle; PSUM accumulation fp32.
"""

import numpy as np
from contextlib import ExitStack

NCORES = 8
BATCH, STATE, FDIM, TIMESPAN = 256, 64, 64, 4096

# per-core tiling
BC = BATCH // NCORES        # 32 batch per core
K = 32                      # steps per block
PAIRS = K // 2              # 16
NB = TIMESPAN // K          # 128 blocks (steps padded 4095 -> 4096)
NBC = 8                     # blocks per chunk
CHUNKS = NB // NBC          # 16
N = NBC * BC                # 256 free-dim per step column
F_COLS = PAIRS * N          # 4096 forcing cols per chunk (parity-stacked)
H_COLS = PAIRS * N          # 4096 prefix cols per chunk
O_COLS = PAIRS * N          # 4096 output cols per chunk (pair-packed)

_NC_CACHE: dict = {}

MM_DTYPE = "bfloat16"


def _set_dims(ncores=8, bc=32, k=32, nbc=8, chunks=16):
    """Override problem dims (testing only). Recomputes derived globals."""
    global NCORES, BATCH, BC, K, PAIRS, NB, NBC, CHUNKS, N
    global F_COLS, H_COLS, O_COLS, TIMESPAN
    NCORES, BC, K, NBC, CHUNKS = ncores, bc, k, nbc, chunks
    BATCH = NCORES * BC
    PAIRS = K // 2
    NB = CHUNKS * NBC
    TIMESPAN = NB * K
    N = NBC * BC
    F_COLS = PAIRS * N
    H_COLS = PAIRS * N
    O_COLS = PAIRS * N


def _build_nc(chunks, nbc, bc, k):
    """Build + compile the per-core Bass module (SPMD: same NEFF all cores)."""
    import concourse.bass as bass  # noqa: F401
    import concourse.tile as tile
    from concourse import bacc, mybir

    pairs = k // 2
    n = nbc * bc
    f_cols = pairs * n
    h_cols = pairs * n
    o_cols = pairs * n
    nb = chunks * nbc
    f32 = mybir.dt.float32
    mdt = getattr(mybir.dt, MM_DTYPE)
    AF = mybir.ActivationFunctionType

    nc = bacc.Bacc("TRN2", target_bir_lowering=False, debug=False)

    f_dram = nc.dram_tensor("f", [128, chunks * f_cols], mdt, kind="ExternalInput")
    w1_dram = nc.dram_tensor("w1", [128, 256], mdt, kind="ExternalInput")
    wpow_dram = nc.dram_tensor("wpow", [128, k * 64], mdt, kind="ExternalInput")
    wp2_dram = nc.dram_tensor("wp2", [128, 128], mdt, kind="ExternalInput")
    bias_dram = nc.dram_tensor("bias", [128, 1], mdt, kind="ExternalInput")
    s0_dram = nc.dram_tensor("s0", [128, (nb + 1) * bc], mdt, kind="ExternalInput")
    out_dram = nc.dram_tensor("out", [128, chunks * o_cols], f32, kind="ExternalOutput")

    with tile.TileContext(nc) as tc, ExitStack() as ctx:
        singles = ctx.enter_context(tc.tile_pool(name="singles", bufs=1))
        fpool = ctx.enter_context(tc.tile_pool(name="fpool", bufs=3))
        hpool = ctx.enter_context(tc.tile_pool(name="hpool", bufs=2))
        opool = ctx.enter_context(tc.tile_pool(name="opool", bufs=3))
        p1ps = ctx.enter_context(tc.tile_pool(name="p1ps", bufs=2, space="PSUM"))
        p3ps = ctx.enter_context(tc.tile_pool(name="p3ps", bufs=3, space="PSUM"))
        p2ps = ctx.enter_context(tc.tile_pool(name="p2ps", bufs=2, space="PSUM"))

        w1 = singles.tile([128, 256], mdt)
        nc.sync.dma_start(out=w1[:], in_=w1_dram[:])
        wpow = singles.tile([128, k * 64], mdt)
        nc.sync.dma_start(out=wpow[:], in_=wpow_dram[:])
        wp2 = singles.tile([128, 128], mdt)
        nc.sync.dma_start(out=wp2[:], in_=wp2_dram[:])
        bias = singles.tile([128, 1], mdt)
        nc.sync.dma_start(out=bias[:], in_=bias_dram[:])
        # block start states: rows 0:64 state (cols 0:bc = x0^T, rest
        # written by P2), rows 64:128 host-zeroed (matmuls read 0:128).
        s_t = singles.tile([128, (nb + 1) * bc], mdt)
        nc.sync.dma_start(out=s_t[:], in_=s0_dram[:])

        w1a = w1[:, 0:128]
        w1b = w1[:, 128:256]
        wp2k = wp2[:, 0:64]     # Wx^K (rows 0:64), zeros below
        wp2i = wp2[:, 64:128]   # [0; I]

        os_pairs = pairs // 2
        os_cols = os_pairs * n

        def emit_p2_step(bg, htile):
            """s_{bg+1} = Wx^K s_bg + h_last(bg's block)."""
            blk = bg % nbc
            ps2 = p2ps.tile([64, bc], f32)
            nc.tensor.matmul(
                ps2[:], wp2k, s_t[:, bg * bc : (bg + 1) * bc],
                start=True, stop=False,
            )
            nc.tensor.matmul(
                ps2[:], wp2i,
                htile[:, (pairs - 1) * n + blk * bc : (pairs - 1) * n + (blk + 1) * bc],
                start=False, stop=True,
            )
            nc.scalar.activation(
                s_t[0:64, (bg + 1) * bc : (bg + 2) * bc], ps2[:], AF.Copy
            )

        def emit_p3(c, htile):
            """x-pairs = wpow-pair . s  +  h-pair; write out."""
            scol = s_t[:, c * n : (c + 1) * n]
            for ohalf in range(2):
                ostage = opool.tile([128, os_cols], f32, tag="OS")
                for pp in range(os_pairs):
                    p = ohalf * os_pairs + pp
                    px = p3ps.tile([128, n], f32)
                    nc.tensor.matmul(
                        px[:], wpow[:, 2 * p * 64 : (2 * p + 2) * 64], scol,
                        start=True, stop=True,
                    )
                    nc.vector.tensor_add(
                        ostage[:, pp * n : (pp + 1) * n], px[:],
                        htile[:, p * n : (p + 1) * n],
                    )
                nc.sync.dma_start(
                    out=out_dram[:, c * o_cols + ohalf * os_cols : c * o_cols + (ohalf + 1) * os_cols],
                    in_=ostage[:],
                )

        htile_prev = None
        for c in range(chunks):
            ftile = fpool.tile([128, f_cols], mdt, tag="F")
            for fh in range(2):
                nc.sync.dma_start(
                    out=ftile[:, fh * (f_cols // 2) : (fh + 1) * (f_cols // 2)],
                    in_=f_dram[:, c * f_cols + fh * (f_cols // 2) : c * f_cols + (fh + 1) * (f_cols // 2)],
                )
            htile = hpool.tile([128, h_cols], mdt, tag="H")

            # P1 pair chain, with prev chunk's P2 steps interleaved so the
            # tensor queue always has stream work between s-chain waits.
            for p in range(pairs):
                ps = p1ps.tile([128, n], f32)
                nc.tensor.matmul(
                    ps[:], w1a, ftile[:, p * n : (p + 1) * n],
                    start=True, stop=(p == 0),
                )
                if p > 0:
                    nc.tensor.matmul(
                        ps[:], w1b, htile[:, (p - 1) * n : p * n],
                        start=False, stop=True,
                    )
                nc.scalar.activation(
                    htile[:, p * n : (p + 1) * n], ps[:], AF.Identity, bias=bias[:, 0:1]
                )
                if c > 0 and p < nbc:
                    emit_p2_step((c - 1) * nbc + p, htile_prev)

            if c > 0:
                emit_p3(c - 1, htile_prev)
            htile_prev = htile

        # epilogue: last chunk's block scan + combine
        for blk in range(nbc):
            emit_p2_step((chunks - 1) * nbc + blk, htile_prev)
        emit_p3(chunks - 1, htile_prev)

    nc.compile()
    return nc


def _get_nc():
    key = (CHUNKS, NBC, BC, K)
    if key not in _NC_CACHE:
        _NC_CACHE[key] = _build_nc(CHUNKS, NBC, BC, K)
    return _NC_CACHE[key]


def _host_prep(inputs, forcing, fc_w, fc_b):
    """Build per-core input maps (numpy only, untimed)."""
    S = STATE
    fc_w = np.asarray(fc_w, np.float32)
    fc_b = np.asarray(fc_b, np.float32)
    Wx = fc_w[:, :S].astype(np.float64)
    Wf = fc_w[:, S:].astype(np.float64)
    b = fc_b.astype(np.float64)

    if MM_DTYPE == "bfloat16":
        import ml_dtypes

        mm_np = ml_dtypes.bfloat16
    else:
        mm_np = np.float32

    # w1: [w1a | w1b] lhsT blocks (out halves: rows 0:64 even, 64:128 odd)
    w1 = np.zeros((128, 256), np.float64)
    w1[0:64, 0:64] = Wf.T            # f_even -> even
    w1[0:64, 64:128] = (Wx @ Wf).T   # f_even -> odd
    w1[64:128, 64:128] = Wf.T        # f_odd  -> odd
    w1[64:128, 128:192] = Wx.T       # h_prev -> even
    w1[64:128, 192:256] = (Wx @ Wx).T  # h_prev -> odd

    # wpow: col block j holds (Wx^{j+1})^T in rows 0:64; rows 64:128 zero
    wpow = np.zeros((128, K * 64), np.float64)
    P = np.eye(S, dtype=np.float64)
    for j in range(K):
        P = Wx @ P
        wpow[0:64, j * 64 : (j + 1) * 64] = P.T
        if j == K - 1:
            wxk = P

    # wp2: [Wx^K | [0; I]] lhsT for the block scan
    wp2 = np.zeros((128, 128), np.float64)
    wp2[0:64, 0:64] = wxk.T
    wp2[64:128, 64:128] = np.eye(64)

    bias128 = np.zeros((128, 1), np.float64)
    bias128[0:64, 0] = b
    bias128[64:128, 0] = Wx @ b + b

    # forcing: [T-1, B, F] -> pad -> [parity*feat, c, p, blk, bfull]
    steps = TIMESPAN
    fpad = np.zeros((steps, BATCH, FDIM), np.float32)
    fpad[: TIMESPAN - 1] = np.asarray(forcing, np.float32)
    # t = (c*NBC + blk)*K + 2p + parity
    arr = fpad.reshape(CHUNKS, NBC, PAIRS, 2, BATCH, FDIM)
    arr = arr.transpose(3, 5, 0, 2, 1, 4)  # [parity, feat, c, p, blk, bfull]

    inputs = np.asarray(inputs, np.float32)
    w1 = w1.astype(mm_np)
    wpow = wpow.astype(mm_np)
    wp2 = wp2.astype(mm_np)
    bias128 = bias128.astype(mm_np)
    in_maps = []
    for core in range(NCORES):
        bs = slice(core * BC, (core + 1) * BC)
        fcore = (
            np.ascontiguousarray(arr[..., bs])
            .reshape(128, CHUNKS * F_COLS)
            .astype(mm_np)
        )
        s0 = np.zeros((128, (NB + 1) * BC), mm_np)
        s0[0:64, 0:BC] = inputs[bs].T.astype(mm_np)
        in_maps.append(
            {
                "f": fcore,
                "w1": w1,
                "wpow": wpow,
                "wp2": wp2,
                "bias": bias128,
                "s0": s0,
            }
        )
    return in_maps


def _host_decode(results, inputs):
    """Per-core out [128, CHUNKS*O_COLS] -> full [T, B, S]."""
    inputs = np.asarray(inputs, np.float32)
    out = np.empty((TIMESPAN, BATCH, STATE), np.float32)
    out[0] = inputs
    for core in range(NCORES):
        o = results[core]["out"].reshape(2, 64, CHUNKS, PAIRS, NBC, BC)
        # [parity, s, c, p, blk, b] -> [c, blk, p, parity, b, s]
        o = o.transpose(2, 4, 3, 0, 5, 1).reshape(TIMESPAN, BC, STATE)
        out[1:, core * BC : (core + 1) * BC] = o[: TIMESPAN - 1]
    return out


def kernel(inputs, forcing, fc_w, fc_b, timespan):
    from concourse.bass_utils import run_bass_kernel_spmd

    timespan = int(timespan)
    assert timespan == TIMESPAN, f"hardcoded for timespan={TIMESPAN}, got {timespan}"
    nc = _get_nc()
    in_maps = _host_prep(inputs, forcing, fc_w, fc_b)
    res = run_bass_kernel_spmd(nc, in_maps, core_ids=list(range(NCORES)))
    return _host_decode(res.results, inputs)


if __name__ == "__main__":
    nc = _get_nc()
    print("built ok")
